# revision 7
# baseline (speedup 1.0000x reference)
"""Trainium2 Bass kernel for nn_CustomConv2D: gather 16x16 patches at given
centers and apply a shared [768 -> 1024] linear projection + bias.

Sharding: data-parallel over batch across 8 NeuronCores (8 images/core,
4608 patches/core); weight replicated. Patch extraction (im2col) runs on
host -- device-side gather via SWDGE indirect-DMA costs ~1.4us/instruction,
~2.4ms for 221k patch rows, far off the roofline -- while the projection
is PE-bound at ~93us/core.

v3 design (from NTFF traces of v1 f32r@129.3us / v2 bf16@142.3us and a
dtype/ordering probe):
 - bf16 matmuls with h-OUTER loop order: 6 consecutive MMs accumulate
   into the same PSUM bank before switching to the other 512-half.
   Probe-measured: bf16 same-bank streams at 216ns/MM (the N=512 floor;
   FWL weight loads 97ns, fully hidden); alternating banks every MM
   (v2) costs 259ns, f32r (v1) is LDW-limited at 227ns.
 - 10 junk warm-up matmuls on zeroed SBUF issue right after the engine
   preamble (~7.4us), before any DMA lands: the PE_HAM clock-gate flips
   to 2.4GHz during the DMA-startup dead time instead of 3.4+us into the
   real stream (v1/v2 lost ~11us to the 1.2GHz cold phase).
 - stall-free dual-ring feed: scalar ring carries the six h0 weight
   pieces then stores; sync ring carries gt block 0, the six h1 weight
   pieces, then growing gt chunks. All bf16 (in 8.6MB, out 9.4MB).
 - bias moved to host (f32 add on the bf16 result); device only
   copy-casts PSUM f32 -> SBUF bf16 on DVE; stores fused 2 blocks/DMA
   (38 total DMA instructions -- the exit barrier's semaphore drain
   scales with DMA count, 14.4us tail in v2).
Accuracy: bf16 in+out, f32 accumulate = 3.0e-3 max-rel vs the 2e-2 gate.
"""

import numpy as np
import ml_dtypes

import concourse.bass as bass  # noqa: F401
from concourse import bacc
import concourse.mybir as mybir
import concourse.tile as tile

# problem shape (hardcoded per contract)
B, C, H, W = 64, 3, 384, 384
N, K, O = 576, 16, 1024
NCORES = 8
B_LOC = B // NCORES          # 8 images per core
NPC = B_LOC * N              # 4608 patches per core
P = 128                      # partitions / patches per block
NBLK = NPC // P              # 36 blocks
KDIM = C * K * K             # 768 contraction dim
KSL = KDIM // P              # 6 k-slices
HALF = O // 2                # 512-wide PSUM-bank-sized output halves
NJUNK = 10                   # HAM warm-up matmuls


def _build(reps: int = 1):
    nc = bacc.Bacc()
    f32 = mybir.dt.float32
    bf16 = mybir.dt.bfloat16

    gt_t = nc.declare_dram_parameter("gt", [P, NBLK, KSL, P], bf16, isOutput=False)
    wt_t = nc.declare_dram_parameter("wt", [P, KSL, O], bf16, isOutput=False)
    out_t = nc.declare_dram_parameter("out", [NBLK // 2, P, 2, O], bf16,
                                      isOutput=True)

    with tile.TileContext(nc) as tc:
        with (
            tc.tile_pool(name="const", bufs=1) as cpool,
            tc.tile_pool(name="osb", bufs=3) as opool,
            tc.tile_pool(name="ps", bufs=3, space="PSUM") as pspool,
            tc.tile_pool(name="junk", bufs=1, space="PSUM") as jpool,
        ):
            wt_sb = cpool.tile([P, KSL, O], bf16)
            gt_sb = cpool.tile([P, NBLK, KSL, P], bf16)

            # HAM warm-up: junk matmuls on zeroed SBUF keep the PE busy
            # through the DMA-startup window so the real stream runs at
            # 2.4GHz from its first matmul.
            junk_a = cpool.tile([P, P], bf16)
            junk_w = cpool.tile([P, HALF], bf16)
            nc.vector.memset(junk_a[:], 0.0)
            nc.vector.memset(junk_w[:], 0.0)
            junk_ps = jpool.tile([P, HALF], f32)
            for _ in range(NJUNK):
                nc.tensor.matmul(junk_ps[:], lhsT=junk_a[:], rhs=junk_w[:],
                                 start=True, stop=True)

            # weight pieces: full-width per-ks slices (2KB/partition DMA
            # lines -- half-width pieces ran descriptor-bound at ~190GB/s
            # and landed after their consuming matmuls). ks0-2 on the
            # scalar ring (free until the first store), gt block 0 then
            # ks3-5 on the sync ring: every piece lands just ahead of
            # block 0's 216ns/MM consumption.
            for ks in range(3):
                nc.scalar.dma_start(wt_sb[:, ks, :], wt_t[:, ks, :])
            nc.sync.dma_start(gt_sb[:, 0], gt_t[:, 0])
            for ks in range(3, KSL):
                nc.sync.dma_start(wt_sb[:, ks, :], wt_t[:, ks, :])
            nc.sync.dma_start(gt_sb[:, 1], gt_t[:, 1])
            nc.sync.dma_start(gt_sb[:, 2:4], gt_t[:, 2:4])
            nc.sync.dma_start(gt_sb[:, 4:8], gt_t[:, 4:8])
            for lo in range(8, NBLK, 8):
                hi = min(lo + 8, NBLK)
                nc.sync.dma_start(gt_sb[:, lo:hi], gt_t[:, lo:hi])

            def body(_i=None):
                for t in range(NBLK):
                    out_ps = pspool.tile([P, O], f32, tag="ps")
                    for h in range(2):
                        hs = slice(h * HALF, (h + 1) * HALF)
                        for ks in range(KSL):
                            nc.tensor.matmul(
                                out_ps[:, hs],
                                lhsT=gt_sb[:, t, ks, :],
                                rhs=wt_sb[:, ks, hs],
                                start=(ks == 0), stop=(ks == KSL - 1),
                            )
                    if t % 2 == 0:
                        o_sb = opool.tile([P, 2, O], bf16, tag="osb")
                    nc.vector.tensor_copy(o_sb[:, t % 2, :], out_ps[:])
                    if t % 2 == 1:
                        nc.scalar.dma_start(out_t[t // 2], o_sb[:])

            if reps == 1:
                body()
            else:
                with tc.For_i(0, reps, 1) as i:
                    body(i)
    nc.finalize()
    return nc


_CACHE = {}


def _get_nc(reps: int = 1):
    if reps not in _CACHE:
        _CACHE[reps] = _build(reps)
    return _CACHE[reps]


def _prep_inputs(x, centers, weight, bias):
    x = np.ascontiguousarray(x, dtype=np.float32)
    centers = np.asarray(centers, dtype=np.int64)
    weight = np.ascontiguousarray(weight, dtype=np.float32)

    # host im2col: patches [B, N, C, K, K]
    win = np.lib.stride_tricks.sliding_window_view(x, (K, K), axis=(2, 3))
    r0 = centers[:, :, 0] - K // 2        # [B, N]
    c0 = centers[:, :, 1] - K // 2
    b_ids = np.arange(B)[:, None]
    patches = win[b_ids, :, r0, c0]       # [B, N, C, K, K]

    # weight [O, C, K, K] -> wT [KDIM, O] -> [128, KSL, O] bf16
    wflat = weight.reshape(O, KDIM)
    wt_host = np.ascontiguousarray(
        wflat.T.reshape(KSL, P, O).transpose(1, 0, 2)).astype(ml_dtypes.bfloat16)

    in_maps = []
    for core in range(NCORES):
        pc = patches[core * B_LOC:(core + 1) * B_LOC].reshape(NPC, KDIM)
        # gt[p, t, ks, n] = patch (t*128+n) element (ks*128+p)
        gt_host = np.ascontiguousarray(
            pc.T.reshape(KSL, P, NBLK, P).transpose(1, 2, 0, 3)
        ).astype(ml_dtypes.bfloat16)
        in_maps.append({"gt": gt_host, "wt": wt_host})
    return in_maps


def kernel(x, centers, weight, bias):
    from concourse.bass_utils import run_bass_kernel_spmd
    nc = _get_nc(1)
    in_maps = _prep_inputs(x, centers, weight, bias)
    res = run_bass_kernel_spmd(nc, in_maps, list(range(NCORES))).results
    # out dram layout [NBLK//2, 128, 2, O] -> [NPC, O]
    out = np.stack(
        [res[i]["out"].transpose(0, 2, 1, 3).reshape(NPC, O).astype(np.float32)
         for i in range(NCORES)], axis=0)
    return (out + np.asarray(bias, dtype=np.float32)).reshape(B, N, O)


# revision 9
# speedup vs baseline: 1.0015x; 1.0015x over previous
"""Trainium2 Bass kernel for nn_CustomConv2D: gather 16x16 patches at given
centers and apply a shared [768 -> 1024] linear projection + bias.

Sharding: data-parallel over batch across 8 NeuronCores (8 images/core,
4608 patches/core); weight replicated. Patch extraction (im2col) runs on
host -- device-side gather via SWDGE indirect-DMA costs ~1.4us/instruction,
~2.4ms for 221k patch rows, far off the roofline -- while the projection
is PE-bound at ~93us/core.

v3 design (from NTFF traces of v1 f32r@129.3us / v2 bf16@142.3us and a
dtype/ordering probe):
 - bf16 matmuls with h-OUTER loop order: 6 consecutive MMs accumulate
   into the same PSUM bank before switching to the other 512-half.
   Probe-measured: bf16 same-bank streams at 216ns/MM (the N=512 floor;
   FWL weight loads 97ns, fully hidden); alternating banks every MM
   (v2) costs 259ns, f32r (v1) is LDW-limited at 227ns.
 - 10 junk warm-up matmuls on zeroed SBUF issue right after the engine
   preamble (~7.4us), before any DMA lands: the PE_HAM clock-gate flips
   to 2.4GHz during the DMA-startup dead time instead of 3.4+us into the
   real stream (v1/v2 lost ~11us to the 1.2GHz cold phase).
 - stall-free dual-ring feed: scalar ring carries the six h0 weight
   pieces then stores; sync ring carries gt block 0, the six h1 weight
   pieces, then growing gt chunks. All bf16 (in 8.6MB, out 9.4MB).
 - bias moved to host (f32 add on the bf16 result); device only
   copy-casts PSUM f32 -> SBUF bf16 on DVE; stores fused 2 blocks/DMA
   (38 total DMA instructions -- the exit barrier's semaphore drain
   scales with DMA count, 14.4us tail in v2).
Accuracy: bf16 in+out, f32 accumulate = 3.0e-3 max-rel vs the 2e-2 gate.
"""

import numpy as np
import ml_dtypes

import concourse.bass as bass  # noqa: F401
from concourse import bacc
import concourse.mybir as mybir
import concourse.tile as tile

# problem shape (hardcoded per contract)
B, C, H, W = 64, 3, 384, 384
N, K, O = 576, 16, 1024
NCORES = 8
B_LOC = B // NCORES          # 8 images per core
NPC = B_LOC * N              # 4608 patches per core
P = 128                      # partitions / patches per block
NBLK = NPC // P              # 36 blocks
KDIM = C * K * K             # 768 contraction dim
KSL = KDIM // P              # 6 k-slices
HALF = O // 2                # 512-wide PSUM-bank-sized output halves
NJUNK = 10                   # HAM warm-up matmuls


def _build(reps: int = 1):
    nc = bacc.Bacc()
    f32 = mybir.dt.float32
    bf16 = mybir.dt.bfloat16

    gt_t = nc.declare_dram_parameter("gt", [P, NBLK, KSL, P], bf16, isOutput=False)
    wt_t = nc.declare_dram_parameter("wt", [P, KSL, O], bf16, isOutput=False)
    out_t = nc.declare_dram_parameter("out", [NBLK // 2, P, 2, O], bf16,
                                      isOutput=True)

    with tile.TileContext(nc) as tc:
        with (
            tc.tile_pool(name="const", bufs=1) as cpool,
            tc.tile_pool(name="osb", bufs=3) as opool,
            tc.tile_pool(name="ps", bufs=3, space="PSUM") as pspool,
            tc.tile_pool(name="junk", bufs=1, space="PSUM") as jpool,
        ):
            wt_sb = cpool.tile([P, KSL, O], bf16)
            gt_sb = cpool.tile([P, NBLK, KSL, P], bf16)

            # HAM warm-up: junk matmuls on zeroed SBUF keep the PE busy
            # through the DMA-startup window so the real stream runs at
            # 2.4GHz from its first matmul.
            junk_a = cpool.tile([P, P], bf16)
            junk_w = cpool.tile([P, HALF], bf16)
            nc.vector.memset(junk_a[:], 0.0)
            nc.vector.memset(junk_w[:], 0.0)
            junk_ps = jpool.tile([P, HALF], f32)
            for _ in range(NJUNK):
                nc.tensor.matmul(junk_ps[:], lhsT=junk_a[:], rhs=junk_w[:],
                                 start=True, stop=True)

            # weight pieces: full-width per-ks slices (2KB/partition DMA
            # lines -- half-width pieces run descriptor-bound and land
            # after their consuming matmuls). Both rings run ~190GB/s and
            # start ~8-9us in; interleave pieces across rings in block 0's
            # consumption order (ks ascending) so each lands just ahead of
            # the 216ns/MM stream: even ks behind gt block 0 on sync, odd
            # ks on scalar (free until the first store).
            for ks in (1, 3, 5):
                nc.scalar.dma_start(wt_sb[:, ks, :], wt_t[:, ks, :])
            nc.sync.dma_start(gt_sb[:, 0], gt_t[:, 0])
            for ks in (0, 2, 4):
                nc.sync.dma_start(wt_sb[:, ks, :], wt_t[:, ks, :])
            nc.sync.dma_start(gt_sb[:, 1], gt_t[:, 1])
            nc.sync.dma_start(gt_sb[:, 2:4], gt_t[:, 2:4])
            nc.sync.dma_start(gt_sb[:, 4:8], gt_t[:, 4:8])
            for lo in range(8, NBLK, 8):
                hi = min(lo + 8, NBLK)
                nc.sync.dma_start(gt_sb[:, lo:hi], gt_t[:, lo:hi])

            def body(_i=None):
                o_sb = None
                for t in range(NBLK):
                    out_ps = pspool.tile([P, O], f32, tag="ps")
                    for h in range(2):
                        hs = slice(h * HALF, (h + 1) * HALF)
                        for ks in range(KSL):
                            nc.tensor.matmul(
                                out_ps[:, hs],
                                lhsT=gt_sb[:, t, ks, :],
                                rhs=wt_sb[:, ks, hs],
                                start=(ks == 0), stop=(ks == KSL - 1),
                            )
                    if t % 2 == 0:
                        o_sb = opool.tile([P, 2, O], bf16, tag="osb")
                    # per-half casts: h0 (ACT) overlaps h1's matmuls; h1
                    # (DVE) is the tail-critical one right after the
                    # block's last matmul.
                    j = t % 2
                    nc.scalar.copy(o_sb[:, j, :HALF], out_ps[:, :HALF])
                    nc.vector.tensor_copy(o_sb[:, j, HALF:], out_ps[:, HALF:])
                    if t == NBLK - 2:
                        nc.scalar.dma_start(out_t[t // 2, :, 0, :],
                                            o_sb[:, 0, :])
                    elif t == NBLK - 1:
                        nc.scalar.dma_start(out_t[t // 2, :, 1, :],
                                            o_sb[:, 1, :])
                    elif t % 2 == 1:
                        nc.scalar.dma_start(out_t[t // 2], o_sb[:])

            if reps == 1:
                body()
            else:
                with tc.For_i(0, reps, 1) as i:
                    body(i)
    nc.finalize()
    return nc


_CACHE = {}


def _get_nc(reps: int = 1):
    if reps not in _CACHE:
        _CACHE[reps] = _build(reps)
    return _CACHE[reps]


def _prep_inputs(x, centers, weight, bias):
    x = np.ascontiguousarray(x, dtype=np.float32)
    centers = np.asarray(centers, dtype=np.int64)
    weight = np.ascontiguousarray(weight, dtype=np.float32)

    # host im2col: patches [B, N, C, K, K]
    win = np.lib.stride_tricks.sliding_window_view(x, (K, K), axis=(2, 3))
    r0 = centers[:, :, 0] - K // 2        # [B, N]
    c0 = centers[:, :, 1] - K // 2
    b_ids = np.arange(B)[:, None]
    patches = win[b_ids, :, r0, c0]       # [B, N, C, K, K]

    # weight [O, C, K, K] -> wT [KDIM, O] -> [128, KSL, O] bf16
    wflat = weight.reshape(O, KDIM)
    wt_host = np.ascontiguousarray(
        wflat.T.reshape(KSL, P, O).transpose(1, 0, 2)).astype(ml_dtypes.bfloat16)

    in_maps = []
    for core in range(NCORES):
        pc = patches[core * B_LOC:(core + 1) * B_LOC].reshape(NPC, KDIM)
        # gt[p, t, ks, n] = patch (t*128+n) element (ks*128+p)
        gt_host = np.ascontiguousarray(
            pc.T.reshape(KSL, P, NBLK, P).transpose(1, 2, 0, 3)
        ).astype(ml_dtypes.bfloat16)
        in_maps.append({"gt": gt_host, "wt": wt_host})
    return in_maps


def kernel(x, centers, weight, bias):
    from concourse.bass_utils import run_bass_kernel_spmd
    nc = _get_nc(1)
    in_maps = _prep_inputs(x, centers, weight, bias)
    res = run_bass_kernel_spmd(nc, in_maps, list(range(NCORES))).results
    # out dram layout [NBLK//2, 128, 2, O] -> [NPC, O]
    out = np.stack(
        [res[i]["out"].transpose(0, 2, 1, 3).reshape(NPC, O).astype(np.float32)
         for i in range(NCORES)], axis=0)
    return (out + np.asarray(bias, dtype=np.float32)).reshape(B, N, O)


# revision 11
# speedup vs baseline: 1.0105x; 1.0090x over previous
"""Trainium2 Bass kernel for nn_CustomConv2D: gather 16x16 patches at given
centers and apply a shared [768 -> 1024] linear projection + bias.

Sharding: data-parallel over batch across 8 NeuronCores (8 images/core,
4608 patches/core); weight replicated. Patch extraction (im2col) runs on
host -- device-side gather via SWDGE indirect-DMA costs ~1.4us/instruction,
~2.4ms for 221k patch rows, far off the roofline -- while the projection
is PE-bound at ~93us/core.

v3 design (from NTFF traces of v1 f32r@129.3us / v2 bf16@142.3us and a
dtype/ordering probe):
 - bf16 matmuls with h-OUTER loop order: 6 consecutive MMs accumulate
   into the same PSUM bank before switching to the other 512-half.
   Probe-measured: bf16 same-bank streams at 216ns/MM (the N=512 floor;
   FWL weight loads 97ns, fully hidden); alternating banks every MM
   (v2) costs 259ns, f32r (v1) is LDW-limited at 227ns.
 - 10 junk warm-up matmuls on zeroed SBUF issue right after the engine
   preamble (~7.4us), before any DMA lands: the PE_HAM clock-gate flips
   to 2.4GHz during the DMA-startup dead time instead of 3.4+us into the
   real stream (v1/v2 lost ~11us to the 1.2GHz cold phase).
 - stall-free dual-ring feed: scalar ring carries the six h0 weight
   pieces then stores; sync ring carries gt block 0, the six h1 weight
   pieces, then growing gt chunks. All bf16 (in 8.6MB, out 9.4MB).
 - bias moved to host (f32 add on the bf16 result); device only
   copy-casts PSUM f32 -> SBUF bf16 on DVE; stores fused 2 blocks/DMA
   (38 total DMA instructions -- the exit barrier's semaphore drain
   scales with DMA count, 14.4us tail in v2).
Accuracy: bf16 in+out, f32 accumulate = 3.0e-3 max-rel vs the 2e-2 gate.
"""

import numpy as np
import ml_dtypes

import concourse.bass as bass  # noqa: F401
from concourse import bacc
import concourse.mybir as mybir
import concourse.tile as tile

# problem shape (hardcoded per contract)
B, C, H, W = 64, 3, 384, 384
N, K, O = 576, 16, 1024
NCORES = 8
B_LOC = B // NCORES          # 8 images per core
NPC = B_LOC * N              # 4608 patches per core
P = 128                      # partitions / patches per block
NBLK = NPC // P              # 36 blocks
KDIM = C * K * K             # 768 contraction dim
KSL = KDIM // P              # 6 k-slices
HALF = O // 2                # 512-wide PSUM-bank-sized output halves
NJUNK = 10                   # HAM warm-up matmuls


def _build(reps: int = 1):
    nc = bacc.Bacc()
    f32 = mybir.dt.float32
    bf16 = mybir.dt.bfloat16

    gt_t = nc.declare_dram_parameter("gt", [P, NBLK, KSL, P], bf16, isOutput=False)
    wt_t = nc.declare_dram_parameter("wt", [P, KSL, O], bf16, isOutput=False)
    out_t = nc.declare_dram_parameter("out", [NBLK // 2, P, 2, O], bf16,
                                      isOutput=True)

    with tile.TileContext(nc) as tc:
        with (
            tc.tile_pool(name="const", bufs=1) as cpool,
            tc.tile_pool(name="osb", bufs=3) as opool,
            tc.tile_pool(name="ps", bufs=3, space="PSUM") as pspool,
            tc.tile_pool(name="junk", bufs=1, space="PSUM") as jpool,
        ):
            wt_sb = cpool.tile([P, KSL, O], bf16)
            gt_sb = cpool.tile([P, NBLK, KSL, P], bf16)

            # HAM warm-up: junk matmuls on zeroed SBUF keep the PE busy
            # through the DMA-startup window so the real stream runs at
            # 2.4GHz from its first matmul.
            junk_a = cpool.tile([P, P], bf16)
            junk_w = cpool.tile([P, HALF], bf16)
            nc.vector.memset(junk_a[:], 0.0)
            nc.vector.memset(junk_w[:], 0.0)
            junk_ps = jpool.tile([P, HALF], f32)
            for _ in range(NJUNK):
                nc.tensor.matmul(junk_ps[:], lhsT=junk_a[:], rhs=junk_w[:],
                                 start=True, stop=True)

            # weight pieces: half-width [ks, h] slices. Both rings move
            # ~150-190GB/s regardless of piece width (measured), so finer
            # pieces post their completion semaphores 2x sooner. Interleave
            # across rings in block 0's consumption order (h outer, ks
            # ascending; even ks behind gt block 0 on the sync ring, odd
            # ks on the scalar ring, which starts ~1us later and is free
            # until the first store) so every piece lands just ahead of
            # the 216ns/MM stream.
            for h in range(2):
                hs = slice(h * HALF, (h + 1) * HALF)
                for ks in (1, 3, 5):
                    nc.scalar.dma_start(wt_sb[:, ks, hs], wt_t[:, ks, hs])
            nc.sync.dma_start(gt_sb[:, 0], gt_t[:, 0])
            for ks in (0, 2, 4):
                nc.sync.dma_start(wt_sb[:, ks, :HALF], wt_t[:, ks, :HALF])
            nc.sync.dma_start(gt_sb[:, 1], gt_t[:, 1])
            for ks in (0, 2, 4):
                nc.sync.dma_start(wt_sb[:, ks, HALF:], wt_t[:, ks, HALF:])
            nc.sync.dma_start(gt_sb[:, 2:4], gt_t[:, 2:4])
            nc.sync.dma_start(gt_sb[:, 4:8], gt_t[:, 4:8])
            for lo in range(8, NBLK, 8):
                hi = min(lo + 8, NBLK)
                nc.sync.dma_start(gt_sb[:, lo:hi], gt_t[:, lo:hi])

            def body(_i=None):
                o_sb = None
                for t in range(NBLK):
                    out_ps = pspool.tile([P, O], f32, tag="ps")
                    for h in range(2):
                        hs = slice(h * HALF, (h + 1) * HALF)
                        for ks in range(KSL):
                            nc.tensor.matmul(
                                out_ps[:, hs],
                                lhsT=gt_sb[:, t, ks, :],
                                rhs=wt_sb[:, ks, hs],
                                start=(ks == 0), stop=(ks == KSL - 1),
                            )
                    if t % 2 == 0:
                        o_sb = opool.tile([P, 2, O], bf16, tag="osb")
                    # per-half casts: h0 (ACT) overlaps h1's matmuls; h1
                    # (DVE) is the tail-critical one right after the
                    # block's last matmul.
                    j = t % 2
                    nc.scalar.copy(o_sb[:, j, :HALF], out_ps[:, :HALF])
                    nc.vector.tensor_copy(o_sb[:, j, HALF:], out_ps[:, HALF:])
                    if t >= NBLK - 2:
                        # tail: store each half as soon as its cast lands
                        # (h0 flies while h1's matmuls still run)
                        nc.scalar.dma_start(out_t[t // 2, :, j, :HALF],
                                            o_sb[:, j, :HALF])
                        nc.scalar.dma_start(out_t[t // 2, :, j, HALF:],
                                            o_sb[:, j, HALF:])
                    elif t % 2 == 1:
                        nc.scalar.dma_start(out_t[t // 2], o_sb[:])

            if reps == 1:
                body()
            else:
                with tc.For_i(0, reps, 1) as i:
                    body(i)
    nc.finalize()
    return nc


_CACHE = {}


def _get_nc(reps: int = 1):
    if reps not in _CACHE:
        _CACHE[reps] = _build(reps)
    return _CACHE[reps]


def _prep_inputs(x, centers, weight, bias):
    x = np.ascontiguousarray(x, dtype=np.float32)
    centers = np.asarray(centers, dtype=np.int64)
    weight = np.ascontiguousarray(weight, dtype=np.float32)

    # host im2col: patches [B, N, C, K, K]
    win = np.lib.stride_tricks.sliding_window_view(x, (K, K), axis=(2, 3))
    r0 = centers[:, :, 0] - K // 2        # [B, N]
    c0 = centers[:, :, 1] - K // 2
    b_ids = np.arange(B)[:, None]
    patches = win[b_ids, :, r0, c0]       # [B, N, C, K, K]

    # weight [O, C, K, K] -> wT [KDIM, O] -> [128, KSL, O] bf16
    wflat = weight.reshape(O, KDIM)
    wt_host = np.ascontiguousarray(
        wflat.T.reshape(KSL, P, O).transpose(1, 0, 2)).astype(ml_dtypes.bfloat16)

    in_maps = []
    for core in range(NCORES):
        pc = patches[core * B_LOC:(core + 1) * B_LOC].reshape(NPC, KDIM)
        # gt[p, t, ks, n] = patch (t*128+n) element (ks*128+p)
        gt_host = np.ascontiguousarray(
            pc.T.reshape(KSL, P, NBLK, P).transpose(1, 2, 0, 3)
        ).astype(ml_dtypes.bfloat16)
        in_maps.append({"gt": gt_host, "wt": wt_host})
    return in_maps


def kernel(x, centers, weight, bias):
    from concourse.bass_utils import run_bass_kernel_spmd
    nc = _get_nc(1)
    in_maps = _prep_inputs(x, centers, weight, bias)
    res = run_bass_kernel_spmd(nc, in_maps, list(range(NCORES))).results
    # out dram layout [NBLK//2, 128, 2, O] -> [NPC, O]
    out = np.stack(
        [res[i]["out"].transpose(0, 2, 1, 3).reshape(NPC, O).astype(np.float32)
         for i in range(NCORES)], axis=0)
    return (out + np.asarray(bias, dtype=np.float32)).reshape(B, N, O)
